# revision 2
# baseline (speedup 1.0000x reference)
"""Causal MHA (B=4, T=2048, D=1024, H=16, Dh=64) on 8 TRN2 NeuronCores.

Sharding: tensor-parallel over heads (2 groups of 8 heads; W_q/W_k/W_v split
column-wise, W_o row-wise) x data-parallel over batch. Core c = (b, g)
computes a partial output for batch b with head-group g; the host sums the
two head-group partials per batch in fp32.

Per-core kernel, all bf16 operands with fp32 PSUM accumulation:
  A: x^T loaded directly via DMA-transpose (bf16), 32 quarter-chunk DMAs.
  B: V natural per t-tile (ones column appended for the softmax denom);
     Q^T/K^T per head-pair via weight-stationary matmuls.
  C: per (pair, q-block of 512): S^T tiles [128k, 512q] computed two heads
     packed (row tiling: head A rows 0-63, head B rows 64-127 of the PE
     array), written into multi-bank PSUM strips; ONE exp per strip (fused
     across k-tiles); causal masking via gpsimd affine_select on the bf16
     probabilities; PV accumulates ctx^T (+denominator row) in PSUM.
     Denominator rows stacked per pair -> one batched DVE reciprocal
     [8, 512]; broadcast via one-hot matmul; normalize on DVE.
     QK matmuls of the next pair are interleaved between attention groups
     to keep the PE dense (HAM stays warm).
  D: out = ctx_n^T.T @ W_o accumulated over 4 pair-chunks, bf16 out.
"""

import numpy as np
import ml_dtypes

import concourse.bass as bass
import concourse.mybir as mybir
import concourse.tile as tile
from concourse import bacc
from concourse.bass_utils import run_bass_kernel_spmd

B, T, D = 4, 2048, 1024
H_TOT, DH = 16, 64
N_CORES = 8
HPC = 8                  # heads per core
NPAIR = 4                # head pairs per core
I = HPC * DH             # 512: inner width per core
BF16 = mybir.dt.bfloat16
F32 = mybir.dt.float32
F32R = mybir.dt.float32r
SCALE = float(DH) ** -0.5
bfnp = ml_dtypes.bfloat16

_NC_CACHE = []


def _emit(nc, tc, ctx):
    x_d = nc.dram_tensor("x", [T, D], BF16, kind="ExternalInput")
    wq_d = nc.dram_tensor("wq", [D, I], BF16, kind="ExternalInput")
    wk_d = nc.dram_tensor("wk", [D, I], BF16, kind="ExternalInput")
    wv_d = nc.dram_tensor("wv", [D, I], BF16, kind="ExternalInput")
    wo_d = nc.dram_tensor("wo", [I, D], BF16, kind="ExternalInput")
    oh_d = nc.dram_tensor("oh", [8, 8 * 128], F32, kind="ExternalInput")
    o_d = nc.dram_tensor("o", [T, D], BF16, kind="ExternalOutput")

    o_view = o_d.ap().rearrange("(n p) d -> n p d", p=128)  # [16,128,1024]

    persist = ctx.enter_context(tc.tile_pool(name="persist", bufs=1))
    ps_strip = ctx.enter_context(
        tc.tile_pool(name="ps_strip", bufs=3, space="PSUM"))
    ps_cps = ctx.enter_context(
        tc.tile_pool(name="ps_cps", bufs=2, space="PSUM"))
    ptpool = ctx.enter_context(tc.tile_pool(name="pt", bufs=6))
    bcspool = ctx.enter_context(tc.tile_pool(name="bcs", bufs=2))
    outpool = ctx.enter_context(tc.tile_pool(name="out_sb", bufs=3))
    stgpool = ctx.enter_context(tc.tile_pool(name="stg", bufs=3))

    _strip_n = [0]

    def strip_tile():
        _strip_n[0] += 1
        return ps_strip.tile([128, 1024], F32, tag="strip",
                             name=f"strip{_strip_n[0]}")

    # persistent SBUF
    xT = [persist.tile([128, T], BF16, tag=f"xT{dc}", name=f"xT{dc}")
          for dc in range(8)]
    qT = [persist.tile([128, T], BF16, tag=f"qT{p}", name=f"qT{p}")
          for p in range(NPAIR)]
    kT = [persist.tile([128, T], BF16, tag=f"kT{p}", name=f"kT{p}")
          for p in range(NPAIR)]
    v3 = [persist.tile([128, HPC, DH + 1], BF16, tag=f"v{t}", name=f"v{t}")
          for t in range(16)]
    ctxu = [persist.tile([128, T], BF16, tag=f"cx{p}", name=f"cx{p}")
            for p in range(NPAIR)]
    den = [persist.tile([8, 512], F32, tag=f"den{p}", name=f"den{p}")
           for p in range(NPAIR)]
    rec = [persist.tile([8, 512], F32R, tag=f"rec{p}", name=f"rec{p}")
           for p in range(NPAIR)]
    den3 = [persist.tile([2, 512], F32, tag=f"d3_{q}", name=f"d3_{q}")
            for q in range(4)]
    rec3 = [persist.tile([2, 512], F32R, tag=f"r3_{q}", name=f"r3_{q}")
            for q in range(4)]
    wq_t = persist.tile([128, 8, I], BF16, tag="wq")
    wk_t = persist.tile([128, 8, I], BF16, tag="wk")
    wv_t = persist.tile([128, 8, I], BF16, tag="wv")
    wo_t = persist.tile([128, 4, D], BF16, tag="wo")
    oh_t = persist.tile([8, 8, 128], F32R, tag="oh")
    ones8 = persist.tile([128, HPC, 1], F32, tag="ones8")

    # ---- DMAs (emission order = rough arrival order) ----
    nc.sync.dma_start(wv_t[:], wv_d.ap().rearrange("(c p) i -> p c i", p=128))
    for tq in range(4):
        for dc in range(8):
            nc.sync.dma_start(
                xT[dc][:, tq * 512:(tq + 1) * 512],
                x_d.ap()[tq * 512:(tq + 1) * 512, dc * 128:(dc + 1) * 128],
                transpose=True,
            )
    nc.sync.dma_start(wq_t[:], wq_d.ap().rearrange("(c p) i -> p c i", p=128))
    nc.sync.dma_start(wk_t[:], wk_d.ap().rearrange("(c p) i -> p c i", p=128))
    nc.sync.dma_start(wo_t[:], wo_d.ap().rearrange("(c p) d -> p c d", p=128))
    nc.sync.dma_start(
        oh_t[:],
        oh_d.ap().bitcast(F32R).rearrange("p (r c) -> p r c", r=8))
    nc.gpsimd.memset(ones8[:], 1.0)

    # ---- stage B1: V natural (+ones col) ----
    for tt in range(16):
        vp = strip_tile()
        for dc in range(8):
            nc.tensor.matmul(
                vp[:, 0:512],
                xT[dc][:, tt * 128:(tt + 1) * 128],
                wv_t[:, dc, :],
                start=(dc == 0), stop=(dc == 7),
            )
        nc.vector.tensor_copy(
            v3[tt][:, :, 0:DH],
            vp[:, 0:512].rearrange("p (h d) -> p h d", h=HPC))
        nc.vector.tensor_copy(v3[tt][:, :, DH:DH + 1], ones8[:])

    # ---- QK chains: generator yielding one (tb, q/k) chain at a time ----
    def qk_chain(p, tb, which):
        w_t, dst = (wq_t, qT) if which == 0 else (wk_t, kT)
        ps = strip_tile()
        for dc in range(8):
            nc.tensor.matmul(
                ps[:, 0:512],
                w_t[:, dc, p * 128:(p + 1) * 128],
                xT[dc][:, tb * 512:(tb + 1) * 512],
                start=(dc == 0), stop=(dc == 7),
            )
        nc.vector.tensor_copy(dst[p][:, tb * 512:(tb + 1) * 512], ps[:, 0:512])

    def emit_qk(p):
        for tb in range(4):
            for which in range(2):
                qk_chain(p, tb, which)

    def qk_iter(p):
        for tb in range(4):
            for which in range(2):
                yield (p, tb, which)

    emit_qk(0)

    # ---- stage C: attention per pair with a filler work-queue ----
    # Filler pieces (QK chains for the next pair, deferred epilogues of the
    # previous pair, out-projection chains) are emitted between attention
    # groups so the PE instruction stream never drains while ACT catches up.
    from collections import deque
    filler = deque()

    def run_filler(n=1):
        for _ in range(n):
            if filler:
                filler.popleft()()

    _bcn = [0]

    def bc_norm(p_, qb_, hl_, oh_ap, rec_ap):
        """Broadcast 1/den onto partitions hl*64.. and normalize ctxu."""
        _bcn[0] += 1
        bcp = strip_tile()
        nc.tensor.matmul(bcp[0:128, 0:512], oh_ap, rec_ap,
                         start=True, stop=True)
        bcs = bcspool.tile([128, 512], F32, tag="bcs", name=f"bcs{_bcn[0]}")
        lo = hl_ * 64
        nc.vector.tensor_copy(bcs[lo:lo + 64, :], bcp[lo:lo + 64, 0:512])
        nc.gpsimd.tensor_mul(
            ctxu[p_][lo:lo + 64, qb_ * 512:(qb_ + 1) * 512],
            ctxu[p_][lo:lo + 64, qb_ * 512:(qb_ + 1) * 512],
            bcs[lo:lo + 64, :])

    def epilogue_pieces(p_):
        """Deferred epilogue for pairs 0..2: one batched recip, then the
        8 broadcast+normalize chains in two bundles."""
        def recip_piece():
            with nc.allow_low_precision(reason="softmax denom recip f32r"):
                nc.vector.reciprocal(rec[p_][:], den[p_][:])

        def norm_piece(qbs):
            def go():
                for qb_ in qbs:
                    for hl_ in range(2):
                        r_ = 2 * qb_ + hl_
                        bc_norm(p_, qb_, hl_, oh_t[:, r_, :], rec[p_][:])
            return go
        return [recip_piece, norm_piece((0, 1)), norm_piece((2, 3))]

    def op_chain(tt):
        osb = outpool.tile([128, 1024], BF16, tag="osb", name=f"osb_{tt}")
        for db in range(2):
            ops = strip_tile()
            for c in range(4):
                nc.tensor.matmul(
                    ops[:, 0:512],
                    ctxu[c][:, tt * 128:(tt + 1) * 128],
                    wo_t[:, c, db * 512:(db + 1) * 512],
                    start=(c == 0), stop=(c == 3),
                )
            if db == 0:
                nc.scalar.copy(osb[:, 0:512], ops[:, 0:512])
            else:
                nc.vector.tensor_copy(osb[:, 512:1024], ops[:, 0:512])
        nc.sync.dma_start(o_view[tt], osb[:])

    for p in range(NPAIR):
        hA, hB = 2 * p, 2 * p + 1
        if p == 0:
            for item in qk_iter(1):
                filler.append(lambda it=item: qk_chain(*it))
        elif p < 3:
            # interleave the deferred epilogue between QK chains so its
            # recip-gated broadcast matmuls never head-block the PE queue
            qks = [lambda it=item: qk_chain(*it) for item in qk_iter(p + 1)]
            eps = epilogue_pieces(p - 1)
            filler.extend([qks[0], qks[1], eps[0], eps[1], qks[2], eps[2]]
                          + qks[3:])
        else:
            filler.extend(epilogue_pieces(2))

        for qb in range(4):
            q0 = qb * 512
            cpsA = ps_cps.tile([DH + 1, 512], F32, tag="cps",
                               name=f"cpsA_{p}_{qb}")
            cpsB = ps_cps.tile([DH + 1, 512], F32, tag="cps",
                               name=f"cpsB_{p}_{qb}")

            # full strips: groups of 2 k-tiles, two heads row-packed
            for g2 in range(2 * qb):
                kt0, kt1 = 2 * g2, 2 * g2 + 1
                sA = strip_tile()
                sB = strip_tile()
                for j, kt in enumerate((kt0, kt1)):
                    k0 = kt * 128
                    off = j * 512
                    nc.tensor.matmul(
                        sA[:, off:off + 512],
                        kT[p][0:64, k0:k0 + 128],
                        qT[p][0:64, q0:q0 + 512],
                        start=True, stop=True,
                    )
                    nc.tensor.matmul(
                        sB[:, off:off + 512],
                        kT[p][64:128, k0:k0 + 128],
                        qT[p][64:128, q0:q0 + 512],
                        start=True, stop=True, tile_position=(64, 0),
                    )
                ptA = ptpool.tile([128, 1024], BF16, tag="pt",
                                  name=f"ptA_{p}_{qb}_{g2}")
                ptB = ptpool.tile([128, 1024], BF16, tag="pt",
                                  name=f"ptB_{p}_{qb}_{g2}")
                nc.scalar.activation(ptA[:, 0:1024], sA[:, 0:1024],
                                     mybir.ActivationFunctionType.Exp,
                                     scale=SCALE)
                nc.scalar.activation(ptB[:, 0:1024], sB[:, 0:1024],
                                     mybir.ActivationFunctionType.Exp,
                                     scale=SCALE)
                run_filler(1)
                for j, kt in enumerate((kt0, kt1)):
                    off = j * 512
                    nc.tensor.matmul(
                        cpsA[:, :], v3[kt][:, hA, :], ptA[:, off:off + 512],
                        start=(kt == 0), stop=False)
                    nc.tensor.matmul(
                        cpsB[:, :], v3[kt][:, hB, :], ptB[:, off:off + 512],
                        start=(kt == 0), stop=False)

            # diagonal: 4 ragged tiles in two 2-bank strips per head
            # strip 1: m0 (w=512) at [0:512], m1 (w=384) at [512:896]
            # strip 2: m3 (w=128) at [0:128], m2 (w=256) at [128:384]
            kt0 = 4 * qb
            d1A = strip_tile()
            d1B = strip_tile()
            for kt, off, w in ((kt0, 0, 512), (kt0 + 1, 512, 384)):
                c0 = (kt - kt0) * 128
                k0 = kt * 128
                nc.tensor.matmul(
                    d1A[:, off:off + w], kT[p][0:64, k0:k0 + 128],
                    qT[p][0:64, q0 + c0:q0 + 512], start=True, stop=True)
                nc.tensor.matmul(
                    d1B[:, off:off + w], kT[p][64:128, k0:k0 + 128],
                    qT[p][64:128, q0 + c0:q0 + 512], start=True, stop=True,
                    tile_position=(64, 0))
            pt1A = ptpool.tile([128, 1024], BF16, tag="pt", name=f"pt1A_{p}_{qb}")
            pt1B = ptpool.tile([128, 1024], BF16, tag="pt", name=f"pt1B_{p}_{qb}")
            nc.scalar.activation(pt1A[:, 0:896], d1A[:, 0:896],
                                 mybir.ActivationFunctionType.Exp, scale=SCALE)
            nc.scalar.activation(pt1B[:, 0:896], d1B[:, 0:896],
                                 mybir.ActivationFunctionType.Exp, scale=SCALE)
            d2A = strip_tile()
            d2B = strip_tile()
            for kt, off, w in ((kt0 + 3, 0, 128), (kt0 + 2, 128, 256)):
                c0 = (kt - kt0) * 128
                k0 = kt * 128
                nc.tensor.matmul(
                    d2A[:, off:off + w], kT[p][0:64, k0:k0 + 128],
                    qT[p][0:64, q0 + c0:q0 + 512], start=True, stop=True)
                nc.tensor.matmul(
                    d2B[:, off:off + w], kT[p][64:128, k0:k0 + 128],
                    qT[p][64:128, q0 + c0:q0 + 512], start=True, stop=True,
                    tile_position=(64, 0))
            pt2A = ptpool.tile([128, 1024], BF16, tag="pt", name=f"pt2A_{p}_{qb}")
            pt2B = ptpool.tile([128, 1024], BF16, tag="pt", name=f"pt2B_{p}_{qb}")
            nc.scalar.activation(pt2A[:, 0:384], d2A[:, 0:384],
                                 mybir.ActivationFunctionType.Exp, scale=SCALE)
            nc.scalar.activation(pt2B[:, 0:384], d2B[:, 0:384],
                                 mybir.ActivationFunctionType.Exp, scale=SCALE)
            for ptx, pty in ((pt1A, pt2A), (pt1B, pt2B)):
                for t_, off, w in ((ptx, 0, 512), (ptx, 512, 384),
                                   (pty, 0, 128), (pty, 128, 256)):
                    nc.gpsimd.affine_select(
                        out=t_[:, off:off + w], in_=t_[:, off:off + w],
                        compare_op=mybir.AluOpType.is_ge, fill=0.0,
                        base=0, pattern=[[1, w]], channel_multiplier=-1,
                    )
            run_filler(1)
            for cps_, pt1_, pt2_, h_ in ((cpsA, pt1A, pt2A, hA),
                                         (cpsB, pt1B, pt2B, hB)):
                for kt, t_, off, w in ((kt0, pt1_, 0, 512),
                                       (kt0 + 1, pt1_, 512, 384),
                                       (kt0 + 3, pt2_, 0, 128),
                                       (kt0 + 2, pt2_, 128, 256)):
                    c0 = (kt - kt0) * 128
                    nc.tensor.matmul(
                        cps_[:, c0:512], v3[kt][:, h_, :], t_[:, off:off + w],
                        start=(kt == 0), stop=(kt == kt0 + 2))

            # denominator rows + unnormalized ctx^T out of PSUM.
            # DVE partition shifts must be multiples of 32, so stage the
            # denominator row at partition 0, then SBUF->SBUF DMA it onto
            # the den stack partition (DMA routes partitions freely).
            # partition-shifted copies must go on DVE (32-aligned crossbar
            # shifts); ScalarE only handles the shift-free cpsA ctx copy.
            for hl, cps_ in ((0, cpsA), (1, cpsB)):
                stg = stgpool.tile([1, 512], F32, tag="stg",
                                   name=f"stg_{p}_{qb}_{hl}")
                nc.vector.tensor_copy(stg[:], cps_[DH:DH + 1, :])
                if p < 3:
                    nc.sync.dma_start(den[p][2 * qb + hl:2 * qb + hl + 1, :],
                                      stg[:])
                else:
                    nc.sync.dma_start(den3[qb][hl:hl + 1, :], stg[:])
            nc.scalar.copy(ctxu[p][0:64, q0:q0 + 512], cpsA[0:DH, :])
            nc.vector.tensor_copy(ctxu[p][64:128, q0:q0 + 512], cpsB[0:DH, :])
            run_filler(1)

            if p == 3:
                # per-qb epilogue + out-projection, deferred through the
                # filler so the recip chain never head-blocks the PE queue
                def ep3(qb_=qb):
                    with nc.allow_low_precision(
                            reason="softmax denom recip f32r"):
                        nc.vector.reciprocal(rec3[qb_][:], den3[qb_][:])
                def nrm3(qb_=qb):
                    for hl in range(2):
                        bc_norm(3, qb_, hl, oh_t[0:2, hl, :], rec3[qb_][:])
                filler.append(ep3)
                filler.append(nrm3)
                for tt in range(4 * qb, 4 * qb + 4):
                    filler.append(lambda t_=tt: op_chain(t_))

    # drain remaining filler (tail out-projection chains)
    while filler:
        run_filler(1)


def _build():
    from contextlib import ExitStack

    nc = bacc.Bacc("TRN2", target_bir_lowering=False, debug=False,
                   enable_asserts=True, num_devices=N_CORES)
    with tile.TileContext(nc) as tc:
        with ExitStack() as ctx:
            _emit(nc, tc, ctx)
    nc.compile()
    return nc


def _get_nc():
    if not _NC_CACHE:
        _NC_CACHE.append(_build())
    return _NC_CACHE[0]


def _onehot():
    oh = np.zeros((8, 8, 128), np.float32)
    for r in range(8):
        hl = r % 2
        oh[r, r, hl * 64:(hl + 1) * 64] = 1.0
    return oh.reshape(8, 8 * 128)


def _in_maps(x, W_q, W_k, W_v, W_o):
    oh = _onehot()
    xb = x.astype(bfnp)
    wqb = W_q.astype(bfnp)
    wkb = W_k.astype(bfnp)
    wvb = W_v.astype(bfnp)
    wob = W_o.astype(bfnp)
    maps = []
    for c in range(N_CORES):
        b, g = c // 2, c % 2
        s = slice(g * I, (g + 1) * I)
        maps.append({
            "x": np.ascontiguousarray(xb[b]),
            "wq": np.ascontiguousarray(wqb[:, s]),
            "wk": np.ascontiguousarray(wkb[:, s]),
            "wv": np.ascontiguousarray(wvb[:, s]),
            "wo": np.ascontiguousarray(wob[s, :]),
            "oh": oh,
        })
    return maps


def kernel(**inputs):
    x = np.asarray(inputs["x"], dtype=np.float32)
    W_q = np.asarray(inputs["W_q"], dtype=np.float32)
    W_k = np.asarray(inputs["W_k"], dtype=np.float32)
    W_v = np.asarray(inputs["W_v"], dtype=np.float32)
    W_o = np.asarray(inputs["W_o"], dtype=np.float32)

    nc = _get_nc()
    res = run_bass_kernel_spmd(nc, _in_maps(x, W_q, W_k, W_v, W_o),
                               core_ids=list(range(N_CORES)))
    out = np.empty((B, T, D), dtype=np.float32)
    for b in range(B):
        out[b] = (res.results[2 * b]["o"].astype(np.float32)
                  + res.results[2 * b + 1]["o"].astype(np.float32))
    return out
